# revision 32
# baseline (speedup 1.0000x reference)
"""Additive-attention fused kernel for one TRN2 chip (8 NeuronCores).

Math (per batch b):
    q = queries @ W_q.T                       [Q, H]
    k = keys    @ W_k.T                       [K, H]
    scores[q,k] = sum_h w_v[h] * tanh(q[q,h] + k[k,h])
    attn = masked_softmax(scores, valid_len)  (mask: k >= L -> weight 0)
    out  = attn @ values                      [Q, V]

Sharding: data-parallel over batch B=4 x 2-way split of Q -> 8 cores,
each core handles [QH=512, :] of one batch.  No collectives needed.

Algorithm (HW-calibrated: ACT ~0.83ns/col + ~0.3us/op, DVE fp16 TT
~0.52ns/col / TS ~0.26ns/col, PE warm 1 col/cycle @2.4GHz, GPSIMD
unusable at ~10us/op):

1. Sparse K: only kb = ceil(max(valid_lens)/128) k-blocks are live; the
   masked tail contributes exactly 0 through the pre-masked [values|1]
   operand, so keys/values beyond kb*128 are never shipped or touched.
2. tanh(z) ~ sum_m c_m sin(pi m z / FL) (odd Fourier series, weighted
   LSQ fit, M=FM=6, |z| <= 2*BCLAMP), which makes scores an accumulated
   matmul over contraction (h, m, {sin,cos}) -- 128 partitions per m.
3. NO explicit clamp: the Sin seeds read the projection PSUM directly;
   the ACT table clamps its argument at +-pi, which clamps sin rows at
   |x|=8 and cos rows at |x|=4 = BCLAMP.  Beyond-4-sigma projections
   are rare enough (~1e-4 of elements, error bounded by one h-term of
   one score) that the output effect is invisible.
4. Chebyshev recurrence for the features: with theta = 2 pi nu1 x,
       t_m = sin(m theta + phase),  t_{m+1} = 2 cos(theta) t_m - t_{m-1}
   so only m=1 needs ACT Sin; every further frequency is 2 DVE
   tensor_tensor ops, narrowed to W2 for m > MLOW (the MLOW banks'
   k-columns are never read there).  The 2cos multiplier for BOTH sides
   comes from t1's own cos rows via partition-shift SBUF DMAs + one DVE
   scale (q cos rows live at 64:128, k cos rows at 0:64).
5. Per-bank frequency stagger: the last two k-banks stop at MLOW
   frequencies, so their exp runs on ACT while DVE still computes the
   remaining recurrence steps; the tail group's exp is per-bank sliced
   so attn@V starts one bank-exp earlier.  m-major emission: every
   score matmul fires the moment its qf_m lands.  A 1-element dummy
   Exp right after the seeds pulls the exp-table load (and its drain)
   into the recurrence window instead of the first real exp.
6. attn@V (all fp16; fp8 fails the accuracy budget -- measured 2.4e-2)
   accumulates per 128-query block with the denominator in an appended
   ones column, j-major across 4 concurrent PSUM accumulators; DVE
   reciprocal + scale produce the fp16 output, DMA'd out in two halves
   on ACT's hwdge queue (on SP they head-block the next iteration's
   input DMAs behind the epilogue).
7. Cross-iteration overlap (the slope-timed number): q/k projections
   share one PSUM tile in the scg0 slot so the next iteration's front
   waits only for this iteration's group-0 exp, not the end-of-
   iteration epilogue; all SBUF pools are double-buffered (bufs=2).
   The staggered-reset stage machinery caps deeper pipelining
   (explicit stage_boundary placements measured strictly worse).

Engine busy per core (kb=6, cost model): DVE ~9.1us, ACT ~9.4us
(incl 2 act-table loads ~2.6us), PE ~11.9us cold / ~6us warm.
HW: ~29.6us/iter slope-timed (baseline 30.0), rel err 1.16e-2
(gate 2e-2; FM=7 fit gives 6.8e-3 at ~+1.5us if margin is ever needed).
"""

import math

import numpy as np

B, QFULL, KK = 4, 1024, 1024
D, H, V = 256, 64, 256
QH = 512            # Q rows per core
NCORES = 8

FM = 6              # number of frequencies
FL = 8.0            # half-period (= 2*BCLAMP so seed args fit the
                    # Sin table domain [-pi, pi] exactly)
BCLAMP = 4.0        # clamp q/k projections to +-BCLAMP
MLOW = 5            # per-bank frequency stagger: the last two k-banks use
                    # MLOW frequencies so their exp hides under the recurrence
QSCALE = 64.0       # q-feature prescale (PSUM holds QSCALE*scores)
EXP_BIAS = -2.0     # softmax shift


def _bank_M(kb):
    if kb <= 4:
        return [FM] * kb
    return [FM if j < kb - 2 else MLOW for j in range(kb)]

_STATE = {}


def _fit_coeffs():
    z = np.linspace(-2 * BCLAMP, 2 * BCLAMP, 4001)
    w = np.exp(-z ** 2 / (2 * 3.2)) + 1e-3
    A = np.sin(np.pi / FL * np.outer(z, np.arange(1, FM + 1)))
    c = np.linalg.lstsq(A * np.sqrt(w)[:, None], np.tanh(z) * np.sqrt(w),
                        rcond=None)[0]
    return c.astype(np.float32)


COEFFS = _fit_coeffs()


def _build_nc(kb, n_iters=1):
    import contextlib
    import concourse.tile as tile
    from concourse import bacc, mybir

    F32 = mybir.dt.float32
    F16 = mybir.dt.float16
    F8 = mybir.dt.float8e4
    Sin = mybir.ActivationFunctionType.Sin
    Exp = mybir.ActivationFunctionType.Exp
    AOp = mybir.AluOpType
    DR = mybir.MatmulPerfMode.DoubleRow
    TWO_PI = 2.0 * math.pi

    KBC = kb * 128                  # live key columns
    W = QH + KBC                    # feature width (q cols | k cols)
    NU1 = 1.0 / (2.0 * FL)
    BANK_M = _bank_M(kb)
    if kb <= 4:
        GROUPS = [(j0, min(j0 + 2, kb)) for j0 in range(0, kb, 2)]
    else:
        # all full-M banks in ONE group (single tail exp call: the per-call
        # overhead beats finer pipelining there), MLOW banks in their own
        # group whose exp hides under the recurrence
        GROUPS = [(0, kb - 2), (kb - 2, kb)]
    NGRP = len(GROUPS)

    nc = bacc.Bacc()
    # host layouts are partition-major so each tensor is ONE DMA:
    # qT [128, 2, QH]: [p, c, :] = queries.T fp16 rows c*128+p
    # wpk [128, 2, 256]: [p, c, 0:128] = wqT2 rows, [.., 128:256] = wkT2
    #   (w*T2 = [D, 128] with the 64 output rows duplicated -> projections
    #    emit all 128 feature partitions directly, no SBUF dup pass)
    qT_d = nc.declare_dram_parameter("queriesT", [128, 2, QH], F16, isOutput=False)
    kT_d = nc.declare_dram_parameter("keysT", [128, 2, KBC], F16, isOutput=False)
    va_d = nc.declare_dram_parameter("vaug", [128, kb, V + 1], F16, isOutput=False)
    wpk_d = nc.declare_dram_parameter("wpk", [128, 2, 256], F16, isOutput=False)
    wvc_d = nc.declare_dram_parameter("wvc", [128, FM], F32, isOutput=False)
    out_d = nc.declare_dram_parameter("out", [QH, V], F16, isOutput=True)

    UNITS = 1 if n_iters == 1 else 2
    trip = n_iters // UNITS
    assert n_iters % UNITS == 0

    with tile.TileContext(nc) as tc:
        with (
            tc.tile_pool(name="singles", bufs=2) as singles,
            tc.tile_pool(name="upool", bufs=2) as upool,
            tc.tile_pool(name="outp", bufs=2) as outp,
            # one PSUM pool, bufs=1: same tag => same bank slot, so the
            # reader/writer chain on each slot hands banks from unit A's
            # tail to unit B's front inside one loop body (no staggered-
            # reset stage gating applies within a body)
            tc.tile_pool(name="ps_big", bufs=1, space="PSUM") as ps_big,
            tc.For_i(0, trip, 1,
                     hint_engines=(mybir.EngineType.PE, mybir.EngineType.DVE,
                                   mybir.EngineType.Activation,
                                   mybir.EngineType.SP, mybir.EngineType.Pool),
                     staggered_reset=True)
            if n_iters > 1 else contextlib.nullcontext(),
        ):
            # ---- per-body constants (shared read-only by both units) ----
            bq = singles.tile([128, 1], F32, tag="bq", name="bq")
            nc.vector.memset(bq[0:H, :], 0.0)
            nc.vector.memset(bq[H:128, :], math.pi / 2)
            bk = singles.tile([128, 1], F32, tag="bk", name="bk")
            nc.vector.memset(bk[0:H, :], math.pi / 2)
            nc.vector.memset(bk[H:128, :], 0.0)
            be = singles.tile([128, 1], F32, tag="be", name="be")
            nc.vector.memset(be, EXP_BIAS)
            t0q = singles.tile([128, 1], F32, tag="t0q", name="t0q")
            nc.vector.memset(t0q[0:H, :], 0.0)
            nc.vector.memset(t0q[H:128, :], 1.0)
            t0k = singles.tile([128, 1], F32, tag="t0k", name="t0k")
            nc.vector.memset(t0k[0:H, :], 1.0)
            nc.vector.memset(t0k[H:128, :], 0.0)
            # dummy Sin prefetches the trig act table at body start
            dum = singles.tile([128, 1], F32, tag="dum", name="dum")
            nc.scalar.activation(dum, bq[:, 0:1], Sin)

            bank_M = BANK_M
            W2 = QH + 128 * ((kb - 2) if kb > 4 else kb)
            nkc = -(-KBC // 512)
            groups = GROUPS
            jmap = {}
            group_M = []
            for g, (j0, j1) in enumerate(groups):
                for j in range(j0, j1):
                    jmap[j] = (g, j - j0)
                group_M.append(max(bank_M[j] for j in range(j0, j1)))
            NG = len(groups)
            # PSUM slot tags: scores group g -> "sc{g}"; psk -> sc0 slot,
            # psq -> sc1 slot (freed by those groups' exps, so unit B's
            # projections start right after unit A's exps); av01 -> the
            # MLOW (last) group's slot; av23 its own pair when kb <= 6
            psk_tag = "sc0"
            psq_tag = "sc1" if NG >= 2 else "psqslot"
            av01_tag = f"sc{NG - 1}" if NG >= 2 else "av01"
            av23_tag = "av23" if kb <= 6 else "sc1"

            st = [dict() for _ in range(UNITS)]

            def emit_front(u):
                s = st[u]
                wpk = singles.tile([128, 2, 256], F16, tag="wpk", name=f"wpk{u}")
                nc.sync.dma_start(wpk, wpk_d[:, :, :])
                qTt = singles.tile([128, 2, QH], F16, tag="qTt", name=f"qTt{u}")
                nc.sync.dma_start(qTt, qT_d[:, :, :])
                kTt = singles.tile([128, 2, KBC], F16, tag="kTt", name=f"kTt{u}")
                nc.sync.dma_start(kTt, kT_d[:, :, :])
                wvc_sb = singles.tile([128, FM], F32, tag="wvc", name=f"wvc{u}")
                nc.sync.dma_start(wvc_sb, wvc_d[:, :])
                v_aug = singles.tile([128, kb, V + 1], F16, tag="vaug",
                                     name=f"vaug{u}")
                nc.sync.dma_start(v_aug, va_d[:, :, :])
                s["wvc"] = wvc_sb
                s["vaug"] = v_aug
                # projections straight into the sc0/sc1 slots (PSUM)
                psk = ps_big.tile([128, nkc * 512], F32, tag=psk_tag,
                                  name=f"psk{u}")
                psq = ps_big.tile([128, 512], F32, tag=psq_tag, name=f"psq{u}")
                for c in range(2):
                    nc.tensor.matmul(psq, wpk[:, c, 0:128], qTt[:, c, :],
                                     start=(c == 0), stop=(c == 1))
                for ci in range(nkc):
                    c0, cw = ci * 512, min(512, KBC - ci * 512)
                    for c in range(2):
                        nc.tensor.matmul(psk[:, c0:c0 + cw], wpk[:, c, 128:256],
                                         kTt[:, c, c0:c0 + cw],
                                         start=(c == 0), stop=(c == 1))
                # m=1 seeds read PSUM directly (ACT table clamps args at
                # +-pi = |x|<=8 sin rows / |x|<=4 cos rows; no DVE clamp)
                t1 = singles.tile([128, W], F16, tag="t1", name=f"t1_{u}")
                nc.scalar.activation(t1[:, 0:QH], psq, Sin,
                                     bias=bq[:, 0:1], scale=TWO_PI * NU1)
                nc.scalar.activation(t1[:, QH:W], psk[:, 0:KBC], Sin,
                                     bias=bk[:, 0:1], scale=TWO_PI * NU1)
                # both-side 2cos from t1's cos rows (q: rows 64:128,
                # k: rows 0:64) via partition-shift DMAs + one DVE scale
                cdup = singles.tile([128, W], F16, tag="cdup", name=f"cdup{u}")
                nc.sync.dma_start(cdup[0:H, 0:QH], t1[H:128, 0:QH])
                nc.sync.dma_start(cdup[H:128, 0:QH], t1[H:128, 0:QH])
                nc.sync.dma_start(cdup[0:H, QH:W], t1[0:H, QH:W])
                nc.sync.dma_start(cdup[H:128, QH:W], t1[0:H, QH:W])
                c2d = singles.tile([128, W], F16, tag="c2d", name=f"c2d{u}")
                nc.vector.tensor_scalar(c2d, cdup, 2.0, None, AOp.mult)
                s["t1"], s["c2d"] = t1, c2d

            def emit_mloop(u):
                """Chebyshev recurrence + qf scales + score matmuls + exps.
                MLOW banks' sweeps are emitted one m-step late so their
                first sweep (gated on the previous unit's epilogue through
                the av01 slot) does not head-block the PE queue."""
                s = st[u]
                wvc_sb, c2d = s["wvc"], s["c2d"]
                scg = [ps_big.tile([128, j1 - j0, QH], F32, tag=f"sc{g}",
                                   name=f"sc{g}_{u}")
                       for g, (j0, j1) in enumerate(groups)]
                p16 = [None] * NG
                qf16 = {}
                tms = {1: s["t1"]}

                def sweep(m, mlow_banks):
                    for j in range(kb):
                        low = bank_M[j] < FM and kb > 4
                        if low != mlow_banks or m > bank_M[j]:
                            continue
                        g, ji = jmap[j]
                        nc.tensor.matmul(
                            scg[g][:, ji, :],
                            tms[m][:, QH + j * 128:QH + (j + 1) * 128],
                            qf16[m], start=(m == 1), stop=(m == bank_M[j]))

                def expg(g):
                    j0, j1 = groups[g]
                    pg = singles.tile([128, j1 - j0, QH], F16, tag=f"p16_{g}",
                                      name=f"p16_{g}_{u}")
                    step = 1 if group_M[g] == FM else 2
                    for s0 in range(0, j1 - j0, step):
                        s1 = min(s0 + step, j1 - j0)
                        nc.scalar.activation(
                            pg[:, s0:s1, :].rearrange("p a b -> p (a b)"),
                            scg[g][:, s0:s1, :].rearrange("p a b -> p (a b)"),
                            Exp, bias=be[:, 0:1], scale=1.0 / QSCALE)
                    p16[g] = pg

                shift = 1 if kb > 4 else 0
                for m in range(1, FM + 1):
                    wm = W if m <= MLOW or kb <= 4 else W2
                    if m >= 2:
                        un = upool.tile([128, W], F16, tag="u", name=f"u{m}_{u}")
                        nc.vector.tensor_tensor(un[:, 0:wm], tms[m - 1][:, 0:wm],
                                                c2d[:, 0:wm], AOp.mult)
                        tm = singles.tile([128, W], F16, tag=f"t{m}",
                                          name=f"t{m}_{u}")
                        if m == 2:
                            nc.vector.tensor_scalar(tm[:, 0:QH], un[:, 0:QH],
                                                    t0q[:, 0:1], None,
                                                    AOp.subtract)
                            nc.vector.tensor_scalar(tm[:, QH:W], un[:, QH:W],
                                                    t0k[:, 0:1], None,
                                                    AOp.subtract)
                        else:
                            nc.vector.tensor_tensor(tm[:, 0:wm], un[:, 0:wm],
                                                    tms[m - 2][:, 0:wm],
                                                    AOp.subtract)
                        tms[m] = tm
                    qf = singles.tile([128, QH], F16, tag=f"qf{m}",
                                      name=f"qf{m}_{u}")
                    if 2 <= m <= 3:
                        nc.scalar.mul(qf, tms[m][:, 0:QH], wvc_sb[:, m - 1:m])
                    else:
                        nc.vector.tensor_scalar(qf, tms[m][:, 0:QH],
                                                wvc_sb[:, m - 1:m], None,
                                                AOp.mult)
                    qf16[m] = qf
                    sweep(m, mlow_banks=False)
                    if shift and m - shift >= 1:
                        sweep(m - shift, mlow_banks=True)
                        if m - shift == MLOW:
                            expg(NG - 1)
                    elif not shift:
                        sweep(m, mlow_banks=True)
                    for g in range(NG):
                        if group_M[g] == m and (not shift or g < NG - 1
                                                or kb <= 4):
                            if not (shift and g == NG - 1):
                                expg(g)
                s["p16"] = p16

            def emit_attnv(u):
                s = st[u]
                p16, v_aug = s["p16"], s["vaug"]
                av01 = ps_big.tile([128, 2, 512], F32, tag=av01_tag,
                                   name=f"av01_{u}")
                av23 = ps_big.tile([128, 2, 512], F32, tag=av23_tag,
                                   name=f"av23_{u}")
                avs = [av01[:, 0, 0:V + 1], av01[:, 1, 0:V + 1],
                       av23[:, 0, 0:V + 1], av23[:, 1, 0:V + 1]]
                jorder = sorted(range(kb), key=lambda j: (bank_M[j], j))
                for idx, j in enumerate(jorder):
                    g, ji = jmap[j]
                    pj = p16[g][:, ji, :]
                    for qb in range(QH // 128):
                        nc.tensor.matmul(avs[qb], pj[:, qb * 128:(qb + 1) * 128],
                                         v_aug[:, j, :], start=(idx == 0),
                                         stop=(idx == kb - 1))
                s["avs"] = avs

            def emit_tail(u, seam=False):
                """Epilogue (DVE reciprocal+scale) + output DMAs.  Mid-body
                tails ride ACT's queue; the body-end tail splits SP/ACT so
                neither next-body queue head-blocks fully."""
                s = st[u]
                avs = s["avs"]
                o16h = [outp.tile([128, 2, V], F16, tag=f"o16_{h}",
                                  name=f"o16_{h}_{u}") for h in range(2)]
                for qb in range(QH // 128):
                    av = avs[qb]
                    rcp = outp.tile([128, 1], F32, tag="rcp", name=f"rcp{qb}_{u}")
                    nc.vector.reciprocal(rcp, av[:, V:V + 1])
                    nc.vector.tensor_scalar(o16h[qb // 2][:, qb % 2, :],
                                            av[:, 0:V], rcp[:, 0:1], None,
                                            AOp.mult)
                    if qb % 2 == 1:
                        eng = (nc.sync if (seam and qb == 1) else nc.scalar)
                        eng.dma_start(
                            out_d.rearrange("(a p) v -> p a v", p=128)
                            [:, (qb - 1):(qb + 1), :], o16h[qb // 2])

            emit_front(0)
            emit_mloop(0)
            for u in range(1, UNITS):
                emit_front(u)
                emit_attnv(u - 1)
                emit_tail(u - 1)
                emit_mloop(u)
            emit_attnv(UNITS - 1)
            emit_tail(UNITS - 1, seam=True)

    nc.finalize()
    return nc


def _build_runner(nc):
    """Cached multi-core PJRT runner (keeps the jitted callable so repeat
    calls don't retrace/recompile)."""
    import jax
    import numpy as _np
    from jax.sharding import Mesh, PartitionSpec
    from jax.experimental.shard_map import shard_map
    from concourse import bass2jax, mybir

    bass2jax.install_neuronx_cc_hook()

    partition_name = nc.partition_id_tensor.name if nc.partition_id_tensor else None
    in_names, out_names, out_avals, zero_outs = [], [], [], []
    for alloc in nc.m.functions[0].allocations:
        if not isinstance(alloc, mybir.MemoryLocationSet):
            continue
        name = alloc.memorylocations[0].name
        if alloc.kind == "ExternalInput":
            if name != partition_name:
                in_names.append(name)
        elif alloc.kind == "ExternalOutput":
            shape = tuple(alloc.tensor_shape)
            dtype = mybir.dt.np(alloc.dtype)
            out_names.append(name)
            out_avals.append(jax.core.ShapedArray(shape, dtype))
            zero_outs.append(_np.zeros(shape, dtype))
    n_params = len(in_names)
    n_outs = len(out_avals)
    all_in_names = list(in_names) + list(out_names)
    if partition_name is not None:
        all_in_names.append(partition_name)
    donate = tuple(range(n_params, n_params + n_outs))

    def _body(*args):
        operands = list(args)
        if partition_name is not None:
            operands.append(bass2jax.partition_id_tensor())
        outs = bass2jax._bass_exec_p.bind(
            *operands,
            out_avals=tuple(out_avals),
            in_names=tuple(all_in_names),
            out_names=tuple(out_names),
            lowering_input_output_aliases=(),
            sim_require_finite=True,
            sim_require_nnan=True,
            nc=nc,
        )
        return tuple(outs)

    devices = jax.devices()[:NCORES]
    assert len(devices) == NCORES, f"need {NCORES} cores, have {len(jax.devices())}"
    mesh = Mesh(_np.asarray(devices), ("core",))
    in_specs = (PartitionSpec("core"),) * (n_params + n_outs)
    out_specs = (PartitionSpec("core"),) * n_outs
    sharded = jax.jit(
        shard_map(_body, mesh=mesh, in_specs=in_specs, out_specs=out_specs,
                  check_rep=False),
        donate_argnums=donate, keep_unused=True)

    def run(in_maps):
        per_core = [[_np.asarray(m[name]) for name in in_names] for m in in_maps]
        concat_in = [
            _np.concatenate([per_core[c][i] for c in range(NCORES)], axis=0)
            for i in range(n_params)
        ]
        concat_zeros = [
            _np.zeros((NCORES * z.shape[0], *z.shape[1:]), z.dtype) for z in zero_outs
        ]
        out_arrs = sharded(*concat_in, *concat_zeros)
        return [
            {
                name: _np.asarray(out_arrs[i]).reshape(NCORES, *out_avals[i].shape)[c]
                for i, name in enumerate(out_names)
            }
            for c in range(NCORES)
        ]

    return run


def get_nc(n_iters=1, kb=None):
    if kb is None:
        kb = _STATE.get("kb", 6)
    key = f"nc{n_iters}_{kb}"
    if key not in _STATE:
        _STATE[key] = _build_nc(kb, n_iters)
    return _STATE[key]


def make_in_maps(queries, keys, values, valid_lens, W_q, W_k, w_v):
    queries = np.asarray(queries, dtype=np.float32)
    keys = np.asarray(keys, dtype=np.float32)
    values = np.asarray(values, dtype=np.float32)
    valid_lens = np.asarray(valid_lens)
    kb = max(1, min(KK // 128, int(-(-int(valid_lens.max()) // 128))))
    _STATE["kb"] = kb
    KBC = kb * 128
    # weights pack: W.T with output rows duplicated (128 feature rows),
    # chunked partition-major: wpk[p, c, 0:128] = wqT2 row c*128+p
    WqT2 = np.concatenate([np.asarray(W_q, np.float32).T] * 2, axis=1)  # [256,128]
    WkT2 = np.concatenate([np.asarray(W_k, np.float32).T] * 2, axis=1)
    wpk = np.concatenate([WqT2, WkT2], axis=1).astype(np.float16)      # [256,256]
    wpk = np.ascontiguousarray(wpk.reshape(2, 128, 256).transpose(1, 0, 2))
    w_v = np.asarray(w_v, dtype=np.float32)
    wv2 = np.concatenate([w_v, w_v])
    wvc = np.ascontiguousarray(wv2[:, None] * COEFFS[None, :] * QSCALE)
    in_maps = []
    for core in range(NCORES):
        b, hf = core // 2, core % 2
        L = int(valid_lens[b])
        mask = (np.arange(KBC) < L).astype(np.float32)[:, None]
        vaug = (np.concatenate([values[b, :KBC], np.ones((KBC, 1), np.float32)],
                               axis=1) * mask).astype(np.float16)
        qT = queries[b, hf * QH:(hf + 1) * QH, :].T.astype(np.float16)  # [256,QH]
        kT = keys[b, :KBC].T.astype(np.float16)                         # [256,KBC]
        in_maps.append({
            "queriesT": np.ascontiguousarray(
                qT.reshape(2, 128, QH).transpose(1, 0, 2)),
            "keysT": np.ascontiguousarray(
                kT.reshape(2, 128, KBC).transpose(1, 0, 2)),
            "vaug": np.ascontiguousarray(
                vaug.reshape(kb, 128, V + 1).transpose(1, 0, 2)),
            "wpk": wpk,
            "wvc": wvc,
        })
    return in_maps


def kernel(queries, keys, values, valid_lens, W_q, W_k, w_v):
    in_maps = make_in_maps(queries, keys, values, valid_lens, W_q, W_k, w_v)
    nc = get_nc()
    rkey = f"run_{_STATE['kb']}"
    if rkey not in _STATE:
        _STATE[rkey] = _build_runner(nc)
    results = _STATE[rkey](in_maps)
    out = np.empty((B, QFULL, V), np.float32)
    for core in range(NCORES):
        b, hf = core // 2, core % 2
        out[b, hf * QH:(hf + 1) * QH, :] = results[core]["out"].astype(np.float32)
    return out

